# revision 1
# baseline (speedup 1.0000x reference)
"""Trainium2 Bass kernel for a dense transformer block (pre-LN, MHA + MLP).

Sharding: data-parallel over batch — B=8 batch elements, one per NeuronCore.
Each core runs the full block on its [1024, 768] slice; no collectives.

Per-core layout scheme:
  - token-major (tm): [tokens(P), features(F)] — used for LN stats, v, residual.
  - feature-major (fm): [features(P), tokens(F)] — used as matmul operands.
  - softmax needs no partition reduction: scores are computed transposed
    (keys on partitions), exp'd, and the key-dim sum comes from an extra
    ones-column appended to v (U^T psum row 64 = sum of exp).
  - matmuls run as float32r (f32 bytes, reduced-precision PE streaming):
    ~213ns stream + ~130ns weight-load per [128x128]x[128x512] matmul, with
    near-fp32 accuracy (measured end-to-end rel l2 err ~1.2e-4).
"""

import os
import sys
from contextlib import ExitStack

import numpy as np

for _p in ("/opt/trn_rl_repo",):
    if os.path.isdir(_p) and _p not in sys.path:
        sys.path.insert(0, _p)

import concourse.bass as bass  # noqa: E402
import concourse.mybir as mybir  # noqa: E402
import concourse.tile as tile  # noqa: E402
from concourse import bacc  # noqa: E402
from concourse.masks import make_identity  # noqa: E402

B, SEQ, C, H, HD, HID = 8, 1024, 768, 12, 64, 3072
P = 128
FP = mybir.dt.float32
FPR = mybir.dt.float32r
TC_N = SEQ // P          # 8 token chunks of 128
NW = 512                 # wide token slice for matmul free dim
NWN = SEQ // NW          # 2
KC = C // P              # 6 contraction chunks over C
QKF = 2 * C // P         # 12 feature chunks covering q then k
HC_N = HID // P          # 24 hidden chunks
PAIRS = H // 2           # 6 head pairs (2 heads share a 128-partition tile)
CS_W = C // 2            # 384-wide output slices for token-major matmuls
SCALE = HD ** -0.5
EPS = 1e-6
AF = mybir.ActivationFunctionType
OP = mybir.AluOpType


def _layer_norm_to_fm(nc, sc, x_big, h_fm, pools, ident, eps_t, g_t, b_t):
    """LN over features of x_big[:, tc, :] (token-major), write transposed
    result into h_fm [P, KC, SEQ] (feature-major) via PE transposes."""
    v, s, te = nc.vector, nc.scalar, nc.tensor
    stat_pool, h_pool, tpsum = pools
    for tcx in range(TC_N):
        xs = x_big[:, tcx, :]
        stats = stat_pool.tile([P, 3, 6], FP, tag="stats", name=f"stats{sc}{tcx}")
        for i in range(3):
            v.bn_stats(out=stats[:, i, :], in_=xs[:, i * 256:(i + 1) * 256])
        mv = stat_pool.tile([P, 2], FP, tag="mv", name=f"mv{sc}{tcx}")
        v.bn_aggr(out=mv, in_=stats)
        rstd = stat_pool.tile([P, 1], FP, tag="rstd", name=f"rstd{sc}{tcx}")
        s.activation(out=rstd, in_=mv[:, 1:2], func=AF.Sqrt, bias=eps_t, scale=1.0)
        v.reciprocal(out=rstd, in_=rstd)
        nb = stat_pool.tile([P, 1], FP, tag="nb", name=f"nb{sc}{tcx}")
        v.tensor_scalar(out=nb, in0=mv[:, 0:1], scalar1=rstd, scalar2=-1.0,
                        op0=OP.mult, op1=OP.mult)
        h_t = h_pool.tile([P, C], FP, tag="h_tm", name=f"htm{sc}{tcx}")
        # apply (x - mu) * rstd on the scalar engine: x*rstd + (-mu*rstd)
        s.activation(out=h_t, in_=xs, func=AF.Identity, bias=nb, scale=rstd)
        if g_t is not None:
            v.tensor_mul(out=h_t, in0=h_t, in1=g_t)
        if b_t is not None:
            v.tensor_add(out=h_t, in0=h_t, in1=b_t)
        for fc in range(KC):
            pst = tpsum.tile([P, P], FP, tag="tp", name=f"tp{sc}{tcx}{fc}")
            te.transpose(pst, h_t[:, fc * P:(fc + 1) * P], ident)
            v.tensor_copy(out=h_fm[:, fc, tcx * P:(tcx + 1) * P], in_=pst)


def _build(ln_affine: bool, proj_bias: bool):
    nc = bacc.Bacc("TRN2", debug=False)
    x_d = nc.dram_tensor("x", [SEQ, C], FP, kind="ExternalInput").ap()
    qkvw_d = nc.dram_tensor("qkv_w", [C, 3 * C], FPR, kind="ExternalInput").ap()
    projw_d = nc.dram_tensor("proj_w", [C, C], FPR, kind="ExternalInput").ap()
    fc1w_d = nc.dram_tensor("fc1_w", [C, HID], FPR, kind="ExternalInput").ap()
    fc2w_d = nc.dram_tensor("fc2_w", [HID, C], FPR, kind="ExternalInput").ap()
    fc1b_d = nc.dram_tensor("fc1_b", [HID], FP, kind="ExternalInput").ap()
    lnp = {}
    if ln_affine:
        for nm in ("ln1_g", "ln1_b", "ln2_g", "ln2_b"):
            lnp[nm] = nc.dram_tensor(nm, [C], FP, kind="ExternalInput").ap()
    if proj_bias:
        lnp["proj_b"] = nc.dram_tensor("proj_b", [C], FP, kind="ExternalInput").ap()
    out_d = nc.dram_tensor("out", [SEQ, C], FP, kind="ExternalOutput").ap()

    with tile.TileContext(nc) as tc:
        with ExitStack() as ctx:
            _body(nc, tc, ctx, x_d, qkvw_d, projw_d, fc1w_d, fc2w_d, fc1b_d,
                  lnp, out_d, ln_affine, proj_bias)
    nc.compile()
    return nc


def _body(nc, tc, ctx, x_d, qkvw_d, projw_d, fc1w_d, fc2w_d, fc1b_d, lnp,
          out_d, ln_affine, proj_bias):
    v, s, te, dma = nc.vector, nc.scalar, nc.tensor, nc.sync

    # ---------- persistent pool ----------
    p0 = ctx.enter_context(tc.tile_pool(name="p0", bufs=1))
    x_tm = p0.tile([P, TC_N, C], FP)       # holds x, then x1, then out
    x_src = x_d.rearrange("(tc p) c -> p tc c", p=P)
    for tcx in range(TC_N):
        dma.dma_start(out=x_tm[:, tcx, :], in_=x_src[:, tcx, :])
    ident = p0.tile([P, P], FP)
    make_identity(nc, ident)
    eps_t = p0.tile([P, 1], FP)
    v.memset(eps_t, EPS)
    ones_f32 = p0.tile([1, 64], FP)
    v.memset(ones_f32, 1.0)
    ones_row = p0.tile([1, 64], FPR)
    v.tensor_copy(out=ones_row, in_=ones_f32)
    ones_col = p0.tile([P, H], FP)
    v.memset(ones_col, 1.0)
    fc1b_t = p0.tile([P, HC_N], FP)
    dma.dma_start(out=fc1b_t, in_=fc1b_d.rearrange("(hc p) -> p hc", p=P))

    def bcast_c(pool, name):
        if name not in lnp:
            return None
        t = pool.tile([P, C], FP, name=name + "_bc", tag=name, bufs=1)
        src = lnp[name]
        ap = bass.AP(tensor=src.tensor, offset=src.offset, ap=[[0, P], src.ap[0]])
        nc.gpsimd.dma_start(out=t, in_=ap)
        return t

    # ---------- stage 1: qkv + attention + proj ----------
    with ExitStack() as s1:
        s1p = s1.enter_context(tc.tile_pool(name="s1", bufs=1))
        q_fm = s1p.tile([P, PAIRS, SEQ], FPR)     # q packed 2 heads/tile
        # k zero-padded per head: full-K=128 scores matmuls keep the whole
        # PE array active (HAM otherwise holds the clock at 1.2 GHz).
        k_pad = s1p.tile([P, H, SEQ], FPR)
        VW = H + 1                                # one pad head slot for U over-read
        v_aug = s1p.tile([P, TC_N, VW * 65], FPR)  # v + ones column per head

        # ----- LN1 -> h_fm, then qkv matmuls -----
        with ExitStack() as sa:
            sap = sa.enter_context(tc.tile_pool(name="sa", bufs=1))
            h_fm = sap.tile([P, KC, SEQ], FPR)
            zz = sap.tile([P, SEQ], FP)
            wqk_pool = sa.enter_context(tc.tile_pool(name="wqk", bufs=3))
            wv_pool = sa.enter_context(tc.tile_pool(name="wv", bufs=2))
            with ExitStack() as ln1:
                stat_pool = ln1.enter_context(tc.tile_pool(name="st1", bufs=4))
                h_pool = ln1.enter_context(tc.tile_pool(name="htm1", bufs=3))
                tpsum = ln1.enter_context(
                    tc.tile_pool(name="tp1", bufs=3, space="PSUM"))
                _layer_norm_to_fm(nc, 1, x_tm, h_fm,
                                  (stat_pool, h_pool, tpsum),
                                  ident, eps_t,
                                  bcast_c(sap, "ln1_g"),
                                  bcast_c(sap, "ln1_b"))

            # attention-operand padding init: emitted after LN1 so it fills
            # DVE idle time during qkv instead of delaying the first transposes
            v.memset(zz, 0.0)
            for h in range(H):
                lo, hi = (64, 128) if h % 2 == 0 else (0, 64)
                v.tensor_copy(out=k_pad[lo:hi, h, :], in_=zz[lo:hi, :])
            for tcx in range(TC_N):
                va = v_aug[:, tcx, :].rearrange("p (h e) -> p h e", e=65)
                v.tensor_copy(out=va[:, 0:H, 64:65], in_=ones_col)
                v.tensor_copy(out=v_aug[:, tcx, H * 65:VW * 65], in_=zz[:, 0:65])

            qkv_r = qkvw_d.rearrange("(kc p) f -> p kc f", p=P)
            qkps = sa.enter_context(tc.tile_pool(name="qkps", bufs=4, space="PSUM"))
            vps = sa.enter_context(tc.tile_pool(name="vps", bufs=2, space="PSUM"))
            for f in range(QKF):
                wqk = wqk_pool.tile([P, KC, P], FPR, tag="wqk", name=f"wqk{f}")
                dma.dma_start(out=wqk, in_=qkv_r[:, :, f * P:(f + 1) * P])
                pss = [qkps.tile([P, NW], FP, tag="qkps", name=f"qkp{f}{nn}")
                       for nn in range(NWN)]
                for kc in range(KC):
                    for nn in range(NWN):  # consecutive matmuls share lhsT
                        te.matmul(pss[nn], lhsT=(wqk[:, kc, :]),
                                  rhs=(h_fm[:, kc, nn * NW:(nn + 1) * NW]),
                                  start=kc == 0, stop=kc == KC - 1)
                for nn in range(NWN):
                    ps = pss[nn]
                    nsl = slice(nn * NW, (nn + 1) * NW)
                    if f < PAIRS:
                        v.tensor_copy(out=q_fm[:, f, nsl], in_=ps)
                    else:
                        pr = f - PAIRS
                        v.tensor_copy(out=k_pad[0:64, 2 * pr, nsl], in_=ps[0:64, :])
                        v.tensor_copy(out=k_pad[64:128, 2 * pr + 1, nsl],
                                      in_=ps[64:128, :])
            wvs = []
            for vs in range(2):
                wv = wv_pool.tile([P, KC, CS_W], FPR, tag="wv", name=f"wv{vs}")
                dma.dma_start(
                    out=wv, in_=qkv_r[:, :, 2 * C + vs * CS_W:2 * C + (vs + 1) * CS_W])
                wvs.append(wv)
            for tcx in range(TC_N):
                pss = [vps.tile([P, CS_W], FP, tag=f"vps{vs}", name=f"vp{tcx}{vs}")
                       for vs in range(2)]
                for kc in range(KC):
                    for vs in range(2):  # consecutive matmuls share lhsT
                        te.matmul(pss[vs],
                                  lhsT=(h_fm[:, kc, tcx * P:(tcx + 1) * P]),
                                  rhs=(wvs[vs][:, kc, :]),
                                  start=kc == 0, stop=kc == KC - 1)
                dst = v_aug[:, tcx, :].rearrange("p (h e) -> p h e", e=65)
                for vs in range(2):
                    v.tensor_copy(out=dst[:, vs * 6:(vs + 1) * 6, 0:64], in_=pss[vs])

        # ----- attention (per head pair) -----
        ap_pool = s1.enter_context(tc.tile_pool(name="ap", bufs=1))
        attn_fm = ap_pool.tile([P, KC, SEQ], FPR)
        pw_pool = s1.enter_context(tc.tile_pool(name="pw", bufs=1))
        pw = pw_pool.tile([P, KC, C], FPR)
        dma.dma_start(out=pw, in_=projw_d.rearrange("(kc p) c -> p kc c", p=P))
        with ExitStack() as sb:
            e_pool = sb.enter_context(tc.tile_pool(name="epool", bufs=3))
            sums_pool = sb.enter_context(tc.tile_pool(name="sums", bufs=2))
            sps = sb.enter_context(tc.tile_pool(name="sps", bufs=2, space="PSUM"))
            ups = sb.enter_context(tc.tile_pool(name="ups", bufs=2, space="PSUM"))
            rps = sb.enter_context(tc.tile_pool(name="rps", bufs=1, space="PSUM"))

            for pr in range(PAIRS):
                for nn in range(NWN):
                    nsl = slice(nn * NW, (nn + 1) * NW)
                    psU = {hh: ups.tile([P, NW], FP, tag=f"u{hh}",
                                        name=f"u{pr}{nn}{hh}") for hh in (0, 1)}
                    ets = {}
                    # 1-iteration skew: U(mc-1) issues after scores(mc) so the
                    # PE never waits on the exp of the chunk it just scored.
                    for mc in range(TC_N + 1):
                        if mc < TC_N:
                            for hh in (0, 1):
                                ps = sps.tile([P, NW], FP, tag="sps",
                                              name=f"sc{pr}{nn}{mc}{hh}")
                                te.matmul(
                                    ps,
                                    lhsT=(k_pad[:, 2 * pr + hh,
                                                mc * P:(mc + 1) * P]),
                                    rhs=(q_fm[:, pr, nsl]),
                                    start=True, stop=True)
                                et = e_pool.tile([P, NW], FPR, tag=f"E{hh}",
                                                 name=f"E{pr}{nn}{mc}{hh}")
                                s.activation(out=et, in_=ps, func=AF.Exp,
                                             scale=SCALE)
                                ets[mc, hh] = et
                        if mc > 0:
                            for hh in (0, 1):
                                ha = 2 * pr + hh
                                te.matmul(
                                    psU[hh],
                                    lhsT=(v_aug[:, mc - 1,
                                                ha * 65:ha * 65 + P]),
                                    rhs=(ets.pop((mc - 1, hh))),
                                    start=mc == 1, stop=mc == TC_N)
                    # normalize: broadcast sums over the 64 head dims via a
                    # K=1 matmul, reciprocal + scale on DVE.
                    rt = sums_pool.tile([P, NW], FP, tag="R", name=f"R{pr}{nn}")
                    for hh in (0, 1):
                        sums = sums_pool.tile([1, NW], FPR, tag=f"sums{hh}",
                                              name=f"sum{pr}{nn}{hh}")
                        s.copy(out=sums, in_=psU[hh][64:65, :])
                        psr = rps.tile([64, NW], FP, tag=f"rps{hh}",
                                       name=f"rp{pr}{nn}{hh}")
                        te.matmul(psr, lhsT=(ones_row), rhs=(sums),
                                  start=True, stop=True)
                        v.reciprocal(out=rt[hh * 64:(hh + 1) * 64, :], in_=psr)
                        v.tensor_mul(
                            out=attn_fm[hh * 64:(hh + 1) * 64, pr, nsl],
                            in0=psU[hh][0:64, :],
                            in1=rt[hh * 64:(hh + 1) * 64, :])

        # ----- proj + residual (into x_tm) -----
        with ExitStack() as sb2:
            pps = sb2.enter_context(tc.tile_pool(name="pps", bufs=3, space="PSUM"))
            pbp = sb2.enter_context(tc.tile_pool(name="pbp", bufs=1))
            projb_t = bcast_c(pbp, "proj_b") if proj_bias else None
            for tcx in range(TC_N):
                pss = [pps.tile([P, CS_W], FP, tag=f"pps{cs}", name=f"pp{tcx}{cs}")
                       for cs in range(2)]
                for kc in range(KC):
                    for cs in range(2):  # consecutive matmuls share lhsT
                        te.matmul(pss[cs],
                                  lhsT=(attn_fm[:, kc, tcx * P:(tcx + 1) * P]),
                                  rhs=(pw[:, kc, cs * CS_W:(cs + 1) * CS_W]),
                                  start=kc == 0, stop=kc == KC - 1)
                for cs in range(2):
                    ps = pss[cs]
                    xsl = x_tm[:, tcx, cs * CS_W:(cs + 1) * CS_W]
                    if projb_t is not None:
                        v.tensor_add(out=ps, in0=ps,
                                     in1=projb_t[:, cs * CS_W:(cs + 1) * CS_W])
                    v.tensor_add(out=xsl, in0=ps, in1=xsl)

    # ---------- stage 2: MLP ----------
    with ExitStack() as s2:
        s2p = s2.enter_context(tc.tile_pool(name="s2", bufs=1))
        g_fm = s2p.tile([P, HC_N, SEQ], FPR)
        w2_pool = s2.enter_context(tc.tile_pool(name="w2", bufs=12))

        with ExitStack() as sc_:
            scp = sc_.enter_context(tc.tile_pool(name="sc", bufs=1))
            h2_fm = scp.tile([P, KC, SEQ], FPR)
            w1_pool = sc_.enter_context(tc.tile_pool(name="w1", bufs=3))
            with ExitStack() as ln2:
                stat_pool = ln2.enter_context(tc.tile_pool(name="st2", bufs=4))
                h_pool = ln2.enter_context(tc.tile_pool(name="htm2", bufs=3))
                tpsum = ln2.enter_context(
                    tc.tile_pool(name="tp2", bufs=3, space="PSUM"))
                _layer_norm_to_fm(nc, 2, x_tm, h2_fm,
                                  (stat_pool, h_pool, tpsum),
                                  ident, eps_t,
                                  bcast_c(scp, "ln2_g"),
                                  bcast_c(scp, "ln2_b"))

            fc1_r = fc1w_d.rearrange("(kc p) f -> p kc f", p=P)
            f1ps = sc_.enter_context(tc.tile_pool(name="f1ps", bufs=3, space="PSUM"))
            for hc in range(HC_N):
                w1 = w1_pool.tile([P, KC, P], FPR, tag="w1", name=f"w1_{hc}")
                dma.dma_start(out=w1, in_=fc1_r[:, :, hc * P:(hc + 1) * P])
                pss = [f1ps.tile([P, NW], FP, tag=f"f1ps{nn}", name=f"f1p{hc}{nn}")
                       for nn in range(NWN)]
                for kc in range(KC):
                    for nn in range(NWN):  # consecutive matmuls share lhsT
                        te.matmul(pss[nn], lhsT=(w1[:, kc, :]),
                                  rhs=(h2_fm[:, kc, nn * NW:(nn + 1) * NW]),
                                  start=kc == 0, stop=kc == KC - 1)
                for nn in range(NWN):
                    s.activation(out=g_fm[:, hc, nn * NW:(nn + 1) * NW], in_=pss[nn],
                                 func=AF.Gelu, bias=fc1b_t[:, hc:hc + 1], scale=1.0)

        # fc2 in groups of 6 hidden chunks, accumulate into x_tm
        GRP = 6
        fc2_r = fc2w_d.rearrange("(hc p) c -> p hc c", p=P)
        out_r = out_d.rearrange("(tc p) c -> p tc c", p=P)
        with ExitStack() as sd:
            f2ps = sd.enter_context(tc.tile_pool(name="f2ps", bufs=3, space="PSUM"))
            for grp in range(HC_N // GRP):
                hcs = list(range(grp * GRP, (grp + 1) * GRP))
                w2t = {}
                for hc in hcs:
                    w2t[hc] = w2_pool.tile([P, C], FPR, tag="w2", name=f"w2_{hc}")
                    dma.dma_start(out=w2t[hc], in_=fc2_r[:, hc, :])
                for tcx in range(TC_N):
                    pss = [f2ps.tile([P, CS_W], FP, tag=f"f2ps{cs}",
                                     name=f"f2p{grp}{tcx}{cs}") for cs in range(2)]
                    for i, hc in enumerate(hcs):
                        for cs in range(2):  # consecutive matmuls share lhsT
                            te.matmul(
                                pss[cs], lhsT=(g_fm[:, hc, tcx * P:(tcx + 1) * P]),
                                rhs=(w2t[hc][:, cs * CS_W:(cs + 1) * CS_W]),
                                start=i == 0, stop=i == GRP - 1)
                    for cs in range(2):
                        xsl = x_tm[:, tcx, cs * CS_W:(cs + 1) * CS_W]
                        v.tensor_add(out=xsl, in0=pss[cs], in1=xsl)
                    if grp == HC_N // GRP - 1:
                        dma.dma_start(out=out_r[:, tcx, :], in_=x_tm[:, tcx, :])


_CACHE = {}
last_results = None


def _get_nc(ln_affine, proj_bias):
    key = (ln_affine, proj_bias)
    if key not in _CACHE:
        _CACHE[key] = _build(*key)
    return _CACHE[key]


def kernel(x, qkv_w, proj_w, proj_b, ln1_g, ln1_b, ln2_g, ln2_b,
           fc1_w, fc1_b, fc2_w, fc2_b):
    global last_results
    from concourse.bass_utils import run_bass_kernel_spmd

    f32 = lambda a: np.ascontiguousarray(np.asarray(a), dtype=np.float32)
    x, qkv_w, proj_w, fc1_w, fc2_w = map(f32, (x, qkv_w, proj_w, fc1_w, fc2_w))
    proj_b, fc1_b, fc2_b = map(f32, (proj_b, fc1_b, fc2_b))
    ln1_g, ln1_b, ln2_g, ln2_b = map(f32, (ln1_g, ln1_b, ln2_g, ln2_b))

    ln_affine = not (np.all(ln1_g == 1) and np.all(ln1_b == 0)
                     and np.all(ln2_g == 1) and np.all(ln2_b == 0))
    proj_bias = bool(np.any(proj_b != 0))
    nc = _get_nc(ln_affine, proj_bias)

    common = {"qkv_w": qkv_w, "proj_w": proj_w, "fc1_w": fc1_w,
              "fc2_w": fc2_w, "fc1_b": fc1_b}
    if ln_affine:
        common.update({"ln1_g": ln1_g, "ln1_b": ln1_b,
                       "ln2_g": ln2_g, "ln2_b": ln2_b})
    if proj_bias:
        common["proj_b"] = proj_b
    in_maps = [dict(common, x=np.ascontiguousarray(x[b])) for b in range(B)]

    res = run_bass_kernel_spmd(nc, in_maps, core_ids=list(range(B)))
    last_results = res
    out = np.stack([r["out"] for r in res.results], axis=0)
    # fc2_b commutes past the final residual add — fold on host.
    return (out + fc2_b[None, None, :]).astype(np.float32)

